# revision 16
# baseline (speedup 1.0000x reference)
"""Mixture-of-Depths layer on 8 Trainium2 NeuronCores.

Strategy: 4 batch rows x 2-way token split of the 512 routed tokens per row.
Host does routing (scores/top-k/sort/gate: 0.04% of FLOPs) and input staging
(transposes/casts/weight folding); each core runs the full decoder block for
its 256 tokens (k/v computed for all 512), plus 1/8 of the aux-loss predictor
GEMM. Device math is bf16 matmuls with f32 accumulation/residuals.
Output assembly (scatter of updated rows into the full tensor) is host-side.
"""

import numpy as np
import ml_dtypes

BF16 = ml_dtypes.bfloat16
F32 = np.float32

B, T, D, F = 4, 4096, 1024, 4096
NH, HD = 16, 64
K = 512            # routed tokens per row (T * 0.125)
OWN = 256          # q-tokens per core
TPC = 2048         # predictor tokens per core (B*T/8)
AUX_W = 0.01
PRED_W = 0.01
EPS = 1e-6

_CACHE = {}


# --------------------------------------------------------------------------
# host staging
# --------------------------------------------------------------------------
def _make_qperm():
    h16 = np.arange(NH)[:, None]
    first = (h16 * HD + np.arange(32)[None, :]).ravel()
    second = (h16 * HD + 32 + np.arange(32)[None, :]).ravel()
    return np.concatenate([first, second])


def _np_bce_mean(x, t):
    x = x.astype(np.float64)
    t = t.astype(np.float64)
    return np.mean(np.maximum(x, 0.0) - x * t + np.log1p(np.exp(-np.abs(x))))


def _stage(inp):
    hs = np.ascontiguousarray(inp['hidden_states'], dtype=F32)
    scores = (hs @ inp['router_w'].astype(F32)).astype(F32)
    idx = np.sort(np.argpartition(scores, T - K, axis=-1)[:, T - K:], axis=-1)
    tgt = np.zeros((B, T), F32)
    np.put_along_axis(tgt, idx, 1.0, axis=1)
    main_aux = _np_bce_mean(scores, tgt) * AUX_W
    gate = (1.0 / (1.0 + np.exp(-np.take_along_axis(scores, idx, axis=1)))).astype(F32)

    ln1 = inp['ln1_g'][:, None].astype(F32)
    ln2 = inp['ln2_g'][:, None].astype(F32)
    qperm = _make_qperm()
    wq_p = np.ascontiguousarray((ln1 * inp['wq'])[:, qperm]).astype(BF16)
    wk_p = np.ascontiguousarray((ln1 * inp['wk'])[:, qperm]).astype(BF16)
    wv_e = (ln1 * inp['wv']).astype(BF16)
    wo_bf = inp['wo'].astype(BF16)
    wg_e = (ln2 * inp['w_gate']).astype(BF16)
    wu_e = (ln2 * inp['w_up']).astype(BF16)
    wd_bf = inp['w_down'].astype(BF16)
    fc1_bf = inp['fc1_w'].astype(BF16)
    hidT_bf = np.ascontiguousarray(hs.reshape(B * T, D).T).astype(BF16)

    inv = (1.0 / (10000.0 ** (np.arange(0, HD, 2, dtype=F32) / HD))).astype(F32)
    ident = np.eye(128, dtype=F32)

    cores = []
    for c in range(8):
        r, h = c // 2, c % 2
        perm = np.arange(K) if h == 0 else np.concatenate(
            [np.arange(OWN, K), np.arange(OWN)])
        posp = idx[r][perm]
        pos_f = posp.astype(F32)
        sel_p = hs[r][posp]                                     # [512, D]
        rs = (1.0 / np.sqrt((sel_p ** 2).mean(axis=1) + EPS)).astype(F32)
        ang = pos_f[None, :] * inv[(np.arange(128) % 32)][:, None]   # [128,512]
        cos_t = (np.cos(ang) * rs[None, :]).astype(F32)
        sin_t = (np.sin(ang) * rs[None, :]).astype(F32)
        rs_col = np.ascontiguousarray(rs.reshape(4, 128).T)     # [128, 4]
        M01 = np.zeros((2, 128, K), F32)
        for q2 in range(2):
            qpos = posp[q2 * 128:(q2 + 1) * 128]
            M01[q2] = (posp[None, :] <= qpos[:, None]).astype(F32)
        tslice = slice(c * TPC, (c + 1) * TPC)
        tt = tgt.reshape(B * T)[tslice]
        cores.append(dict(
            hidT=np.ascontiguousarray(hidT_bf[:, tslice]),
            selT=np.ascontiguousarray(sel_p.T).astype(BF16),
            selTo=np.ascontiguousarray(sel_p[:OWN].T).astype(F32),
            cosT=cos_t, sinT=sin_t, rscol=rs_col,
            m01=np.ascontiguousarray(M01).astype(BF16),
            ttarg=np.ascontiguousarray(tt.reshape(16, 128).T).astype(F32),
            gown=gate[r][perm[:OWN]].reshape(1, OWN).astype(F32),
            fc1b=np.ascontiguousarray(
                np.broadcast_to(inp['fc1_b'].astype(F32), (128, 256))),
            fc2w=np.ascontiguousarray(
                np.broadcast_to(inp['fc2_w'], (128, 256))).astype(BF16),
            fc2b=np.full((128, 1), np.float32(inp['fc2_b']), F32),
            identbf=ident.astype(BF16), identf=ident,
            wq=wq_p, wk=wk_p, wv=wv_e, wo=wo_bf,
            wg=wg_e, wu=wu_e, wd=wd_bf, fc1=fc1_bf,
        ))
        cores[-1]['_own_tok'] = posp[:OWN]
        cores[-1]['_row'] = r
    return cores, main_aux, hs


# --------------------------------------------------------------------------
# device program
# --------------------------------------------------------------------------
def _build():
    import concourse.bass as bass
    import concourse.bacc as bacc
    import concourse.mybir as mybir
    import concourse.tile as tile
    from concourse.alu_op_type import AluOpType as op
    dt = mybir.dt
    AF = mybir.ActivationFunctionType
    AX = mybir.AxisListType.X
    SafeTileContext = tile.TileContext

    nc = bacc.Bacc("TRN2")
    I = {}
    def di(name, shape, d=dt.bfloat16):
        I[name] = nc.dram_tensor(name, shape, d, kind="ExternalInput")
        return I[name]

    hidT = di('hidT', [D, TPC])
    fc1 = di('fc1', [D, 256])
    fc1b = di('fc1b', [128, 256], dt.float32)
    fc2w = di('fc2w', [128, 256])
    fc2b = di('fc2b', [128, 1], dt.float32)
    ttarg = di('ttarg', [128, 16], dt.float32)
    selT = di('selT', [D, K])
    selTo = di('selTo', [D, OWN], dt.float32)
    cosT = di('cosT', [128, K], dt.float32)
    sinT = di('sinT', [128, K], dt.float32)
    rscol = di('rscol', [128, 4], dt.float32)
    m01 = di('m01', [2, 128, K])
    gown = di('gown', [1, OWN], dt.float32)
    identbf = di('identbf', [128, 128])
    identf = di('identf', [128, 128], dt.float32)
    wq = di('wq', [D, D]); wk = di('wk', [D, D])
    wv = di('wv', [D, D]); wo = di('wo', [D, D])
    wg = di('wg', [D, F]); wu = di('wu', [D, F])
    wd = di('wd', [F, D])

    upd = nc.dram_tensor('upd', [OWN, D], dt.float32, kind="ExternalOutput")
    bce = nc.dram_tensor('bce', [128, 1], dt.float32, kind="ExternalOutput")

    f32, bf16 = dt.float32, dt.bfloat16
    f32r = dt.float32r

    with SafeTileContext(nc) as tc:
        import contextlib
        ctx = contextlib.ExitStack()
        with ctx:
            const = ctx.enter_context(tc.tile_pool(name="const", bufs=1))
            sb = ctx.enter_context(tc.tile_pool(name="sb", bufs=2))
            wpool = ctx.enter_context(tc.tile_pool(name="wpool", bufs=2))
            stream = ctx.enter_context(tc.tile_pool(name="stream", bufs=2))
            ps = ctx.enter_context(tc.tile_pool(name="ps", bufs=4, space="PSUM"))
            psacc = ctx.enter_context(tc.tile_pool(name="psacc", bufs=2, space="PSUM"))

            # ---- constants / small loads ----
            selT_t = const.tile([128, 8, K], bf16)
            nc.sync.dma_start(out=selT_t, in_=selT[:, :].rearrange("(a p) t -> p a t", p=128))
            selTo_t = const.tile([128, 8, OWN], f32)
            nc.sync.dma_start(out=selTo_t, in_=selTo[:, :].rearrange("(a p) t -> p a t", p=128))
            cos_t = const.tile([128, K], f32)
            nc.sync.dma_start(out=cos_t, in_=cosT[:, :])
            sin_t = const.tile([128, K], f32)
            nc.sync.dma_start(out=sin_t, in_=sinT[:, :])
            rs_t = const.tile([128, 4], f32)
            nc.sync.dma_start(out=rs_t, in_=rscol[:, :])
            m01_t = const.tile([128, 2, K], bf16)
            nc.sync.dma_start(out=m01_t, in_=m01[:, :, :].rearrange("q p t -> p q t"))
            fc1b_t = const.tile([128, 256], f32)
            nc.sync.dma_start(out=fc1b_t, in_=fc1b[:, :])
            fc2w_t = const.tile([128, 256], bf16)
            nc.sync.dma_start(out=fc2w_t, in_=fc2w[:, :])
            fc2b_t = const.tile([128, 1], f32)
            nc.sync.dma_start(out=fc2b_t, in_=fc2b[:, :])
            ttarg_t = const.tile([128, 16], f32)
            nc.sync.dma_start(out=ttarg_t, in_=ttarg[:, :])
            idbf_t = const.tile([128, 128], bf16)
            nc.sync.dma_start(out=idbf_t, in_=identbf[:, :])
            idf_t = const.tile([128, 128], f32)
            nc.sync.dma_start(out=idf_t, in_=identf[:, :])
            g_ap = gown[:, :]
            gbc_t = const.tile([128, OWN], f32)
            nc.sync.dma_start(out=gbc_t, in_=bass.AP(
                tensor=g_ap.tensor, offset=g_ap.offset, ap=[[0, 128], g_ap.ap[-1]]))
            eps_t = const.tile([128, 1], f32)
            nc.vector.memset(eps_t, EPS)
            ones_f = const.tile([128, 128], f32)
            nc.vector.memset(ones_f, 1.0)
            ones_t = const.tile([128, 128], f32r)
            nc.vector.tensor_copy(out=ones_t, in_=ones_f)

            # ---- predictor: fc1 -> gelu -> fc2 -> bce ----
            fc1_t = const.tile([128, 8, 256], bf16)
            nc.sync.dma_start(out=fc1_t, in_=fc1[:, :].rearrange("(a p) n -> p a n", p=128))
            logit_t = const.tile([128, 16], f32)
            for grp in range(4):
                hid_t = stream.tile([128, 8, 512], bf16, tag="big")
                for kt in range(8):
                    nc.sync.dma_start(
                        out=hid_t[:, kt, :],
                        in_=hidT[kt * 128:(kt + 1) * 128, grp * 512:(grp + 1) * 512])
                for mi in range(4):
                    mt = grp * 4 + mi
                    pp = ps.tile([128, 256], f32, tag="ps")
                    for kt in range(8):
                        nc.tensor.matmul(pp, hid_t[:, kt, mi * 128:(mi + 1) * 128],
                                         fc1_t[:, kt, :], start=(kt == 0), stop=(kt == 7))
                    pre = sb.tile([128, 256], f32, tag="pred_pre")
                    nc.vector.tensor_tensor(out=pre, in0=pp, in1=fc1b_t, op=op.add)
                    gel = sb.tile([128, 256], bf16, tag="pred_gel")
                    nc.scalar.activation(out=gel, in_=pre, func=AF.Gelu_apprx_tanh)
                    fm = sb.tile([128, 256], f32, tag="pred_fm")
                    nc.vector.tensor_tensor(out=fm, in0=gel, in1=fc2w_t, op=op.mult)
                    nc.vector.tensor_reduce(out=logit_t[:, mt:mt + 1], in_=fm, axis=AX, op=op.add)
            # logits += fc2b ; bce = relu(x) - x*t + softplus(-|x|)
            nc.vector.tensor_scalar(logit_t, logit_t, fc2b_t[:, 0:1], None, op0=op.add)
            xt_t = const.tile([128, 16], f32)
            nc.vector.tensor_tensor(out=xt_t, in0=logit_t, in1=ttarg_t, op=op.mult)
            r0_t = const.tile([128, 16], f32)
            nc.vector.scalar_tensor_tensor(out=r0_t, in0=logit_t, scalar=0.0,
                                           in1=xt_t, op0=op.max, op1=op.subtract)
            ab_t = const.tile([128, 16], f32)
            nc.scalar.activation(out=ab_t, in_=logit_t, func=AF.Abs)
            ex_t = const.tile([128, 16], f32)
            nc.scalar.activation(out=ex_t, in_=ab_t, func=AF.Exp, scale=-1.0)
            sp_t = const.tile([128, 16], f32)
            nc.scalar.activation(out=sp_t, in_=ex_t, func=AF.Ln, bias=1.0)
            be_t = const.tile([128, 16], f32)
            nc.vector.tensor_tensor(out=be_t, in0=r0_t, in1=sp_t, op=op.add)
            bce_t = const.tile([128, 1], f32)
            nc.vector.tensor_reduce(out=bce_t, in_=be_t, axis=AX, op=op.add)
            nc.sync.dma_start(out=bce[:, :], in_=bce_t)

            # ---- q/k projections + rope ----
            wq_t = wpool.tile([128, 8, D], bf16, tag="wmat")
            nc.sync.dma_start(out=wq_t, in_=wq[:, :].rearrange("(a p) m -> p a m", p=128))
            wk_t = wpool.tile([128, 8, D], bf16, tag="wmat")
            nc.sync.dma_start(out=wk_t, in_=wk[:, :].rearrange("(a p) m -> p a m", p=128))
            q_bf = const.tile([128, 8, OWN], bf16)
            k_bf = const.tile([128, 8, K], bf16)

            def qk_proj(w_t, out_t, n):
                for i in range(4):
                    pa = ps.tile([128, n], f32, tag="ps")
                    pb = ps.tile([128, n], f32, tag="ps")
                    for kt in range(8):
                        nc.tensor.matmul(pa, w_t[:, kt, i * 128:(i + 1) * 128],
                                         selT_t[:, kt, 0:n], start=(kt == 0), stop=(kt == 7))
                    for kt in range(8):
                        nc.tensor.matmul(pb, w_t[:, kt, (i + 4) * 128:(i + 5) * 128],
                                         selT_t[:, kt, 0:n], start=(kt == 0), stop=(kt == 7))
                    t0 = sb.tile([128, n], f32, tag="rope0")
                    t1 = sb.tile([128, n], f32, tag="rope1")
                    nc.vector.tensor_tensor(out=t0, in0=pa, in1=cos_t[:, 0:n], op=op.mult)
                    nc.vector.tensor_tensor(out=t1, in0=pb, in1=sin_t[:, 0:n], op=op.mult)
                    nc.vector.tensor_tensor(out=out_t[:, i, :], in0=t0, in1=t1, op=op.subtract)
                    t2 = sb.tile([128, n], f32, tag="rope0")
                    t3 = sb.tile([128, n], f32, tag="rope1")
                    nc.vector.tensor_tensor(out=t2, in0=pb, in1=cos_t[:, 0:n], op=op.mult)
                    nc.vector.tensor_tensor(out=t3, in0=pa, in1=sin_t[:, 0:n], op=op.mult)
                    nc.vector.tensor_tensor(out=out_t[:, i + 4, :], in0=t2, in1=t3, op=op.add)

            qk_proj(wq_t, q_bf, OWN)
            qk_proj(wk_t, k_bf, K)

            # ---- v projection (token-major) ----
            wv_t = wpool.tile([128, 8, D], bf16, tag="wmat")
            nc.sync.dma_start(out=wv_t, in_=wv[:, :].rearrange("(a p) m -> p a m", p=128))
            v_bf = const.tile([128, 4, D], bf16)
            for j in range(4):
                for half in range(2):
                    pv = ps.tile([128, 512], f32, tag="ps")
                    for kt in range(8):
                        nc.tensor.matmul(pv, selT_t[:, kt, j * 128:(j + 1) * 128],
                                         wv_t[:, kt, half * 512:(half + 1) * 512],
                                         start=(kt == 0), stop=(kt == 7))
                    nc.vector.tensor_scalar(v_bf[:, j, half * 512:(half + 1) * 512],
                                            pv, rs_t[:, j:j + 1], None, op0=op.mult)

            # ---- attention ----
            attn_bf = const.tile([128, 8, OWN], bf16)
            for hp in range(8):
                po = ps.tile([128, OWN], f32, tag="ps")
                for sub in range(2):
                    hh = 2 * hp + sub
                    rstrip, tb = hh % 4, hh // 4
                    pt_bf = sb.tile([128, 4, 2, 128], bf16, tag="ptb")
                    for qt in range(2):
                        s_ps = ps.tile([128, K], f32, tag="ps")
                        lo = 32 * rstrip
                        nc.tensor.matmul(
                            s_ps, q_bf[lo:lo + 32, tb, qt * 128:(qt + 1) * 128],
                            k_bf[lo:lo + 32, tb, :], start=True, stop=False,
                            tile_position=(lo, 0))
                        nc.tensor.matmul(
                            s_ps, q_bf[lo:lo + 32, tb + 4, qt * 128:(qt + 1) * 128],
                            k_bf[lo:lo + 32, tb + 4, :], start=False, stop=True,
                            tile_position=(lo, 0))
                        p_bf = sb.tile([128, K], bf16, tag="pbf")
                        nc.scalar.activation(out=p_bf, in_=s_ps, func=AF.Exp, scale=0.125)
                        nc.vector.tensor_tensor(out=p_bf, in0=p_bf, in1=m01_t[:, qt, :], op=op.mult)
                        ssum = sb.tile([128, 1], f32, tag="ssum")
                        nc.vector.tensor_reduce(out=ssum, in_=p_bf, axis=AX, op=op.add)
                        rcp = sb.tile([128, 1], f32, tag="rcp")
                        nc.vector.reciprocal(out=rcp, in_=ssum)
                        nc.vector.tensor_scalar(p_bf, p_bf, rcp[:, 0:1], None, op0=op.mult)
                        for j in range(4):
                            ptp = ps.tile([128, 128], bf16, tag="ps")
                            nc.tensor.transpose(ptp, p_bf[:, j * 128:(j + 1) * 128], idbf_t)
                            nc.vector.tensor_copy(out=pt_bf[:, j, qt, :], in_=ptp)
                    for j in range(4):
                        nc.tensor.matmul(po[64 * sub:64 * sub + 64, :],
                                         v_bf[:, j, 64 * hh:64 * hh + 64],
                                         pt_bf[:, j, :, :].rearrange("p a b -> p (a b)"),
                                         start=(j == 0), stop=(j == 3),
                                         tile_position=(0, 64 * sub))
                nc.vector.tensor_copy(out=attn_bf[:, hp, :], in_=po)

            # ---- wo + residual + rmsnorm2 ----
            wo_t = wpool.tile([128, 8, D], bf16, tag="wmat")
            nc.sync.dma_start(out=wo_t, in_=wo[:, :].rearrange("(a p) m -> p a m", p=128))
            e_t = const.tile([128, 8, OWN], f32)     # attn block output (pre-residual)
            h_t = const.tile([128, 8, OWN], f32)
            pss = psacc.tile([128, OWN], f32, tag="psacc")
            for mt in range(8):
                ph = ps.tile([128, OWN], f32, tag="ps")
                for kt in range(8):
                    nc.tensor.matmul(ph, wo_t[:, kt, mt * 128:(mt + 1) * 128],
                                     attn_bf[:, kt, :], start=(kt == 0), stop=(kt == 7))
                nc.vector.tensor_copy(out=e_t[:, mt, :], in_=ph)
                nc.vector.tensor_tensor(out=h_t[:, mt, :], in0=ph, in1=selTo_t[:, mt, :], op=op.add)
                sq = sb.tile([128, OWN], f32r, tag="sq")
                nc.vector.tensor_tensor(out=sq, in0=h_t[:, mt, :],
                                        in1=h_t[:, mt, :], op=op.mult)
                nc.tensor.matmul(pss, ones_t[:, :], sq,
                                 start=(mt == 0), stop=(mt == 7))
            ln_t = const.tile([128, OWN], f32)
            nc.scalar.activation(out=ln_t, in_=pss, func=AF.Ln,
                                 bias=eps_t[:, 0:1], scale=1.0 / D)
            rs2_t = const.tile([128, OWN], f32)
            nc.scalar.activation(out=rs2_t, in_=ln_t, func=AF.Exp, scale=-0.5)
            m_bf = const.tile([128, 8, OWN], bf16)
            for mt in range(8):
                nc.vector.tensor_tensor(out=m_bf[:, mt, :], in0=h_t[:, mt, :],
                                        in1=rs2_t, op=op.mult)

            # ---- MLP: gate (silu) then up (prod), then down ----
            sg_t = const.tile([128, 32, OWN], bf16)
            for og in range(8):
                ch = stream.tile([128, 8, 512], bf16, tag="big")
                nc.sync.dma_start(out=ch, in_=wg[:, og * 512:(og + 1) * 512]
                                  .rearrange("(a p) m -> p a m", p=128))
                for o4 in range(4):
                    pg = ps.tile([128, OWN], f32, tag="ps")
                    for kt in range(8):
                        nc.tensor.matmul(pg, ch[:, kt, o4 * 128:(o4 + 1) * 128],
                                         m_bf[:, kt, :], start=(kt == 0), stop=(kt == 7))
                    nc.scalar.activation(out=sg_t[:, og * 4 + o4, :], in_=pg, func=AF.Silu)
            for og in range(8):
                ch = stream.tile([128, 8, 512], bf16, tag="big")
                nc.sync.dma_start(out=ch, in_=wu[:, og * 512:(og + 1) * 512]
                                  .rearrange("(a p) m -> p a m", p=128))
                for o4 in range(4):
                    pu = ps.tile([128, OWN], f32, tag="ps")
                    for kt in range(8):
                        nc.tensor.matmul(pu, ch[:, kt, o4 * 128:(o4 + 1) * 128],
                                         m_bf[:, kt, :], start=(kt == 0), stop=(kt == 7))
                    ot = og * 4 + o4
                    ub = sb.tile([128, OWN], bf16, tag="ub")
                    nc.vector.tensor_copy(out=ub, in_=pu)
                    nc.vector.tensor_tensor(out=sg_t[:, ot, :], in0=ub,
                                            in1=sg_t[:, ot, :], op=op.mult)
            for mt in range(8):
                wdm = stream.tile([128, 32, 128], bf16, tag="big")
                nc.sync.dma_start(out=wdm, in_=wd[:, mt * 128:(mt + 1) * 128]
                                  .rearrange("(a p) m -> p a m", p=128))
                pd = psacc.tile([128, OWN], f32, tag="psacc")
                for kt2 in range(32):
                    nc.tensor.matmul(pd, wdm[:, kt2, :], sg_t[:, kt2, :],
                                     start=(kt2 == 0), stop=(kt2 == 31))
                t1 = sb.tile([128, OWN], f32, tag="cmb1")
                nc.vector.tensor_tensor(out=t1, in0=pd, in1=e_t[:, mt, :], op=op.add)
                t2 = sb.tile([128, OWN], f32, tag="cmb2")
                nc.vector.tensor_tensor(out=t2, in0=t1, in1=gbc_t, op=op.mult)
                uT = sb.tile([128, OWN], f32, tag="cmb3")
                nc.vector.tensor_tensor(out=uT, in0=t2, in1=selTo_t[:, mt, :], op=op.add)
                updo = sb.tile([128, 2, 128], f32, tag="updo")
                for qt in range(2):
                    pt = ps.tile([128, 128], f32, tag="ps")
                    nc.tensor.transpose(pt, uT[:, qt * 128:(qt + 1) * 128], idf_t)
                    nc.vector.tensor_copy(out=updo[:, qt, :], in_=pt)
                nc.sync.dma_start(
                    out=upd[:, mt * 128:(mt + 1) * 128]
                    .rearrange("(q p) c -> p q c", p=128),
                    in_=updo)
    nc.compile()
    return nc


# --------------------------------------------------------------------------
# entry point
# --------------------------------------------------------------------------
def kernel(**inputs):
    from concourse.bass_utils import run_bass_kernel_spmd

    cores, main_aux, hs = _stage(inputs)
    if 'nc' not in _CACHE:
        _CACHE['nc'] = _build()
    nc = _CACHE['nc']

    names = ['hidT', 'fc1', 'fc1b', 'fc2w', 'fc2b', 'ttarg', 'selT', 'selTo',
             'cosT', 'sinT', 'rscol', 'm01', 'gown', 'identbf', 'identf',
             'wq', 'wk', 'wv', 'wo', 'wg', 'wu', 'wd']
    in_maps = [{n: ci[n] for n in names} for ci in cores]
    res = run_bass_kernel_spmd(nc, in_maps, core_ids=list(range(8)),
                               trace=bool(int(__import__('os').environ.get('KERNEL_TRACE', '0'))))
    _CACHE['last_result'] = res

    out = hs.copy()
    bce_total = 0.0
    for c in range(8):
        r = cores[c]['_row']
        out[r][cores[c]['_own_tok']] = res.results[c]['upd']
        bce_total += float(res.results[c]['bce'].sum())
    total_aux = np.float32(main_aux + (bce_total / (B * T)) * PRED_W)
    return out, total_aux


# revision 39
# speedup vs baseline: 1.2371x; 1.2371x over previous
"""Mixture-of-Depths layer on 8 Trainium2 NeuronCores.

Strategy: 4 batch rows x 2-way token split of the 512 routed tokens per row.
Host does routing (scores/top-k/sort/gate: 0.04% of FLOPs) and input staging
(transposes/casts/weight folding); each core runs the full decoder block for
its 256 tokens (k/v computed for all 512), plus 1/8 of the aux-loss predictor
GEMM. Device math is bf16 matmuls with f32 accumulation/residuals.
Output assembly (scatter of updated rows into the full tensor) is host-side.
"""

import numpy as np
import ml_dtypes

BF16 = ml_dtypes.bfloat16
F32 = np.float32

B, T, D, F = 4, 4096, 1024, 4096
NH, HD = 16, 64
K = 512            # routed tokens per row (T * 0.125)
OWN = 256          # q-tokens per core
TPC = 2048         # predictor tokens per core (B*T/8)
AUX_W = 0.01
PRED_W = 0.01
EPS = 1e-6

_CACHE = {}


# --------------------------------------------------------------------------
# host staging
# --------------------------------------------------------------------------
def _make_qperm():
    h16 = np.arange(NH)[:, None]
    first = (h16 * HD + np.arange(32)[None, :]).ravel()
    second = (h16 * HD + 32 + np.arange(32)[None, :]).ravel()
    return np.concatenate([first, second])


def _np_bce_mean(x, t):
    x = x.astype(np.float64)
    t = t.astype(np.float64)
    return np.mean(np.maximum(x, 0.0) - x * t + np.log1p(np.exp(-np.abs(x))))


def _stage(inp):
    hs = np.ascontiguousarray(inp['hidden_states'], dtype=F32)
    scores = (hs @ inp['router_w'].astype(F32)).astype(F32)
    idx = np.sort(np.argpartition(scores, T - K, axis=-1)[:, T - K:], axis=-1)
    tgt = np.zeros((B, T), F32)
    np.put_along_axis(tgt, idx, 1.0, axis=1)
    main_aux = _np_bce_mean(scores, tgt) * AUX_W
    gate = (1.0 / (1.0 + np.exp(-np.take_along_axis(scores, idx, axis=1)))).astype(F32)

    ln1 = inp['ln1_g'][:, None].astype(F32)
    ln2 = inp['ln2_g'][:, None].astype(F32)
    qperm = _make_qperm()
    wq_p = np.ascontiguousarray((ln1 * inp['wq'])[:, qperm]).astype(BF16)
    wk_p = np.ascontiguousarray((ln1 * inp['wk'])[:, qperm]).astype(BF16)
    wv_e = (ln1 * inp['wv']).astype(BF16)
    wo_bf = inp['wo'].astype(BF16)
    wg_e = (ln2 * inp['w_gate']).astype(BF16)
    wu_e = (ln2 * inp['w_up']).astype(BF16)
    wd_bf = inp['w_down'].astype(BF16)
    fc1_bf = inp['fc1_w'].astype(BF16)
    hidT_bf = np.ascontiguousarray(hs.reshape(B * T, D).T).astype(BF16)

    inv = (1.0 / (10000.0 ** (np.arange(0, HD, 2, dtype=F32) / HD))).astype(F32)
    ident = np.eye(128, dtype=F32)

    cores = []
    for c in range(8):
        r, h = c // 2, c % 2
        perm = np.arange(K) if h == 0 else np.concatenate(
            [np.arange(OWN, K), np.arange(OWN)])
        posp = idx[r][perm]
        pos_f = posp.astype(F32)
        sel_p = hs[r][posp]                                     # [512, D]
        rs = (1.0 / np.sqrt((sel_p ** 2).mean(axis=1) + EPS)).astype(F32)
        ang = pos_f[None, :] * inv[(np.arange(128) % 32)][:, None]   # [128,512]
        cos_t = (np.cos(ang) * rs[None, :]).astype(F32)
        sin_t = (np.sin(ang) * rs[None, :]).astype(F32)
        rs_col = np.ascontiguousarray(rs.reshape(4, 128).T)     # [128, 4]
        # transposed mask tiles: m01T[j, p, q] = 1 if pos[kt=j*128+p] <= pos[qt=q]
        M01T = (posp[:, None] <= posp[None, :OWN]).astype(F32).reshape(4, 128, OWN)
        tslice = slice(c * TPC, (c + 1) * TPC)
        tt = tgt.reshape(B * T)[tslice]
        cores.append(dict(
            hidT=np.ascontiguousarray(hidT_bf[:, tslice]),
            selT=np.ascontiguousarray(sel_p.T).astype(BF16),
            selTo=np.ascontiguousarray(sel_p[:OWN].T).astype(F32),
            cosT=cos_t, sinT=sin_t, rscol=rs_col,
            m01=np.ascontiguousarray(M01T).astype(BF16),
            ttarg=np.ascontiguousarray(tt.reshape(16, 128).T).astype(F32),
            gown=gate[r][perm[:OWN]].reshape(1, OWN).astype(F32),
            fc1b=np.ascontiguousarray(
                np.broadcast_to(inp['fc1_b'].astype(F32), (128, 256))),
            fc2w=np.ascontiguousarray(
                np.broadcast_to(inp['fc2_w'], (128, 256))).astype(BF16),
            fc2b=np.full((128, 1), np.float32(inp['fc2_b']), F32),
            identf=ident,
            wq=wq_p, wk=wk_p, wv=wv_e, wo=wo_bf,
            wg=wg_e, wu=wu_e, wd=wd_bf, fc1=fc1_bf,
        ))
        cores[-1]['_own_tok'] = posp[:OWN]
        cores[-1]['_row'] = r
    return cores, main_aux, hs


# --------------------------------------------------------------------------
# device program
# --------------------------------------------------------------------------
def _build():
    import concourse.bass as bass
    import concourse.bacc as bacc
    import concourse.mybir as mybir
    import concourse.tile as tile
    from concourse.alu_op_type import AluOpType as op
    dt = mybir.dt
    AF = mybir.ActivationFunctionType
    AX = mybir.AxisListType.X
    SafeTileContext = tile.TileContext

    nc = bacc.Bacc("TRN2")
    I = {}
    def di(name, shape, d=dt.bfloat16):
        I[name] = nc.dram_tensor(name, shape, d, kind="ExternalInput")
        return I[name]

    hidT = di('hidT', [D, TPC])
    fc1 = di('fc1', [D, 256])
    fc1b = di('fc1b', [128, 256], dt.float32)
    fc2w = di('fc2w', [128, 256])
    fc2b = di('fc2b', [128, 1], dt.float32)
    ttarg = di('ttarg', [128, 16], dt.float32)
    selT = di('selT', [D, K])
    selTo = di('selTo', [D, OWN], dt.float32)
    cosT = di('cosT', [128, K], dt.float32)
    sinT = di('sinT', [128, K], dt.float32)
    rscol = di('rscol', [128, 4], dt.float32)
    m01 = di('m01', [4, 128, OWN])
    gown = di('gown', [1, OWN], dt.float32)
    identf = di('identf', [128, 128], dt.float32)
    wq = di('wq', [D, D]); wk = di('wk', [D, D])
    wv = di('wv', [D, D]); wo = di('wo', [D, D])
    wg = di('wg', [D, F]); wu = di('wu', [D, F])
    wd = di('wd', [F, D])

    upd = nc.dram_tensor('upd', [OWN, D], dt.float32, kind="ExternalOutput")
    bce = nc.dram_tensor('bce', [128, 1], dt.float32, kind="ExternalOutput")

    f32, bf16 = dt.float32, dt.bfloat16
    f32r = dt.float32r

    with SafeTileContext(nc) as tc:
        import contextlib
        ctx = contextlib.ExitStack()
        with ctx:
            const = ctx.enter_context(tc.tile_pool(name="const", bufs=1))
            sb = ctx.enter_context(tc.tile_pool(name="sb", bufs=2))
            wpool = ctx.enter_context(tc.tile_pool(name="wpool", bufs=2))
            stream = ctx.enter_context(tc.tile_pool(name="stream", bufs=2))
            ps = ctx.enter_context(tc.tile_pool(name="ps", bufs=6, space="PSUM"))
            psacc = ctx.enter_context(tc.tile_pool(name="psacc", bufs=2, space="PSUM"))

            # ---- constants / small loads ----
            selT_t = const.tile([128, 8, K], bf16, tag="selT_sg")
            nc.sync.dma_start(out=selT_t, in_=selT[:, :].rearrange("(a p) t -> p a t", p=128))
            selTo_t = const.tile([128, 8, OWN], f32)
            nc.sync.dma_start(out=selTo_t, in_=selTo[:, :].rearrange("(a p) t -> p a t", p=128))
            cos_t = const.tile([128, K], f32)
            nc.sync.dma_start(out=cos_t, in_=cosT[:, :])
            sin_t = const.tile([128, K], f32)
            nc.sync.dma_start(out=sin_t, in_=sinT[:, :])
            rs_t = const.tile([128, 4], f32)
            nc.sync.dma_start(out=rs_t, in_=rscol[:, :])
            m01_t = const.tile([128, 4, OWN], bf16)
            nc.sync.dma_start(out=m01_t, in_=m01[:, :, :].rearrange("j p q -> p j q"))
            fc1b_t = const.tile([128, 256], f32)
            nc.sync.dma_start(out=fc1b_t, in_=fc1b[:, :])
            fc2w_t = const.tile([128, 256], bf16)
            nc.sync.dma_start(out=fc2w_t, in_=fc2w[:, :])
            fc2b_t = const.tile([128, 1], f32)
            nc.sync.dma_start(out=fc2b_t, in_=fc2b[:, :])
            ttarg_t = const.tile([128, 16], f32)
            nc.sync.dma_start(out=ttarg_t, in_=ttarg[:, :])
            onesb_t = const.tile([128, 128], bf16)
            nc.vector.memset(onesb_t, 1.0)
            idf_t = const.tile([128, 128], f32)
            nc.sync.dma_start(out=idf_t, in_=identf[:, :])
            g_ap = gown[:, :]
            gbc_t = const.tile([128, OWN], f32)
            nc.sync.dma_start(out=gbc_t, in_=bass.AP(
                tensor=g_ap.tensor, offset=g_ap.offset, ap=[[0, 128], g_ap.ap[-1]]))
            eps_t = const.tile([128, 1], f32)
            nc.vector.memset(eps_t, EPS)
            ones_f = const.tile([128, 128], f32)
            nc.vector.memset(ones_f, 1.0)
            ones_t = const.tile([128, 128], f32r)
            nc.vector.tensor_copy(out=ones_t, in_=ones_f)

            # ---- predictor pieces (emitted interleaved with attention) ----
            fc1_t = const.tile([128, 8, 256], bf16)
            nc.sync.dma_start(out=fc1_t, in_=fc1[:, :].rearrange("(a p) n -> p a n", p=128))
            logit_t = const.tile([128, 16], f32)

            def pred_group(grp):
                hid_t = stream.tile([128, 8, 512], bf16, tag="wdh", bufs=2)
                nc.gpsimd.dma_start(
                    out=hid_t,
                    in_=hidT[:, grp * 512:(grp + 1) * 512]
                    .rearrange("(a p) t -> p a t", p=128))
                for mi in range(4):
                    mt = grp * 4 + mi
                    pp = ps.tile([128, 256], f32, tag="ps")
                    for kt in range(8):
                        nc.tensor.matmul(pp, hid_t[:, kt, mi * 128:(mi + 1) * 128],
                                         fc1_t[:, kt, :], start=(kt == 0), stop=(kt == 7))
                    pre = sb.tile([128, 256], f32, tag="pred_pre")
                    nc.vector.tensor_tensor(out=pre, in0=pp, in1=fc1b_t, op=op.add)
                    gel = sb.tile([128, 256], bf16, tag="pred_gel")
                    nc.scalar.activation(out=gel, in_=pre, func=AF.Gelu_apprx_tanh)
                    fm = sb.tile([128, 256], f32, tag="pred_fm")
                    nc.vector.tensor_tensor(out=fm, in0=gel, in1=fc2w_t, op=op.mult)
                    nc.vector.tensor_reduce(out=logit_t[:, mt:mt + 1], in_=fm,
                                            axis=AX, op=op.add)

            def pred_tail():
                # logits += fc2b ; bce = relu(x) - x*t + log1p(exp(-|x|))
                nc.vector.tensor_scalar(logit_t, logit_t, fc2b_t[:, 0:1], None, op0=op.add)
                xt_t = const.tile([128, 16], f32)
                nc.vector.tensor_tensor(out=xt_t, in0=logit_t, in1=ttarg_t, op=op.mult)
                r0_t = const.tile([128, 16], f32)
                nc.vector.scalar_tensor_tensor(out=r0_t, in0=logit_t, scalar=0.0,
                                               in1=xt_t, op0=op.max, op1=op.subtract)
                ab_t = const.tile([128, 16], f32)
                nc.scalar.activation(out=ab_t, in_=logit_t, func=AF.Abs)
                ex_t = const.tile([128, 16], f32)
                nc.scalar.activation(out=ex_t, in_=ab_t, func=AF.Exp, scale=-1.0)
                sp_t = const.tile([128, 16], f32)
                nc.scalar.activation(out=sp_t, in_=ex_t, func=AF.Ln, bias=1.0)
                be_t = const.tile([128, 16], f32)
                nc.vector.tensor_tensor(out=be_t, in0=r0_t, in1=sp_t, op=op.add)
                bce_t = const.tile([128, 1], f32)
                nc.vector.tensor_reduce(out=bce_t, in_=be_t, axis=AX, op=op.add)
                nc.sync.dma_start(out=bce[:, :], in_=bce_t)

            # ---- q/k projections + rope ----
            wq_t = wpool.tile([128, 8, D], bf16, tag="wmat")
            nc.sync.dma_start(out=wq_t, in_=wq[:, :].rearrange("(a p) m -> p a m", p=128))
            wk_t = wpool.tile([128, 8, D], bf16, tag="wmat")
            nc.sync.dma_start(out=wk_t, in_=wk[:, :].rearrange("(a p) m -> p a m", p=128))
            q_bf = const.tile([128, 8, OWN], bf16)
            k_bf = const.tile([128, 8, K], bf16)

            def qk_proj(w_t, out_t, n):
                for i in range(4):
                    pa = ps.tile([128, n], f32, tag="ps")
                    pb = ps.tile([128, n], f32, tag="ps")
                    for kt in range(8):
                        nc.tensor.matmul(pa, w_t[:, kt, i * 128:(i + 1) * 128],
                                         selT_t[:, kt, 0:n], start=(kt == 0), stop=(kt == 7))
                    for kt in range(8):
                        nc.tensor.matmul(pb, w_t[:, kt, (i + 4) * 128:(i + 5) * 128],
                                         selT_t[:, kt, 0:n], start=(kt == 0), stop=(kt == 7))
                    t0 = sb.tile([128, n], f32, tag="rope0")
                    t1 = sb.tile([128, n], f32, tag="rope1")
                    nc.vector.tensor_tensor(out=t0, in0=pa, in1=cos_t[:, 0:n], op=op.mult)
                    nc.vector.tensor_tensor(out=t1, in0=pb, in1=sin_t[:, 0:n], op=op.mult)
                    nc.gpsimd.tensor_tensor(out=out_t[:, i, :], in0=t0, in1=t1, op=op.subtract)
                    t2 = sb.tile([128, n], f32, tag="rope0")
                    t3 = sb.tile([128, n], f32, tag="rope1")
                    nc.vector.tensor_tensor(out=t2, in0=pb, in1=cos_t[:, 0:n], op=op.mult)
                    nc.vector.tensor_tensor(out=t3, in0=pa, in1=sin_t[:, 0:n], op=op.mult)
                    nc.gpsimd.tensor_tensor(out=out_t[:, i + 4, :], in0=t2, in1=t3, op=op.add)

            qk_proj(wq_t, q_bf, OWN)
            pred_group(0)
            qk_proj(wk_t, k_bf, K)
            pred_group(1)

            # ---- v projection (token-major) ----
            wv_t = wpool.tile([128, 8, D], bf16, tag="wmat")
            nc.sync.dma_start(out=wv_t, in_=wv[:, :].rearrange("(a p) m -> p a m", p=128))
            v_bf = const.tile([128, 4, D], bf16)
            for j in range(4):
                for half in range(2):
                    pv = ps.tile([128, 512], f32, tag="ps")
                    for kt in range(8):
                        nc.tensor.matmul(pv, selT_t[:, kt, j * 128:(j + 1) * 128],
                                         wv_t[:, kt, half * 512:(half + 1) * 512],
                                         start=(kt == 0), stop=(kt == 7))
                    nc.vector.tensor_scalar(v_bf[:, j, half * 512:(half + 1) * 512],
                                            pv, rs_t[:, j:j + 1], None, op0=op.mult)
                if j == 1:
                    pred_group(2)
            pred_group(3)
            pred_tail()

            # ---- attention, S computed transposed: P_T[kt, qt] ----
            # exp -> mask (DVE) -> row-sums via ones-matmul (PE) -> PV directly
            # from P_T (no PE transposes); normalization folded into the
            # PSUM->SBUF copy of the PV output (free-dim op).
            attn_bf = const.tile([128, 8, OWN], bf16)
            for hp in range(8):
                po = ps.tile([128, OWN], f32, tag="ps")
                rcps = []
                for sub in range(2):
                    hh = 2 * hp + sub
                    rstrip, tb = hh % 4, hh // 4
                    lo = 32 * rstrip
                    pt = sb.tile([128, 4, OWN], bf16, tag="pbf", bufs=4)
                    for half in range(2):
                        st = ps.tile([128, 2, OWN], f32, tag="ps")
                        for jj in range(2):
                            j = half * 2 + jj
                            nc.tensor.matmul(
                                st[:, jj, :],
                                k_bf[lo:lo + 32, tb, j * 128:(j + 1) * 128],
                                q_bf[lo:lo + 32, tb, :], start=True, stop=False,
                                tile_position=(lo, 0))
                            nc.tensor.matmul(
                                st[:, jj, :],
                                k_bf[lo:lo + 32, tb + 4, j * 128:(j + 1) * 128],
                                q_bf[lo:lo + 32, tb + 4, :], start=False, stop=True,
                                tile_position=(lo, 0))
                        nc.scalar.activation(out=pt[:, half * 2:half * 2 + 2, :],
                                             in_=st, func=AF.Exp, scale=0.125)
                        nc.vector.tensor_tensor(
                            out=pt[:, half * 2:half * 2 + 2, :],
                            in0=pt[:, half * 2:half * 2 + 2, :],
                            in1=m01_t[:, half * 2:half * 2 + 2, :], op=op.mult)
                    pssum = ps.tile([128, OWN], f32, tag="ps")
                    for j in range(4):
                        nc.tensor.matmul(pssum, onesb_t, pt[:, j, :],
                                         start=(j == 0), stop=(j == 3))
                    rcpb = sb.tile([128, OWN], f32, tag="rcp", bufs=3)
                    nc.vector.reciprocal_approx_fast(out=rcpb, in_=pssum)
                    rcps.append(rcpb)
                    for j in range(4):
                        nc.tensor.matmul(po[64 * sub:64 * sub + 64, :],
                                         v_bf[:, j, 64 * hh:64 * hh + 64],
                                         pt[:, j, :],
                                         start=(j == 0), stop=(j == 3),
                                         tile_position=(0, 64 * sub))
                for sub in range(2):
                    sl = slice(64 * sub, 64 * sub + 64)
                    nc.vector.tensor_tensor(out=attn_bf[sl, hp, :], in0=po[sl, :],
                                            in1=rcps[sub][sl, :], op=op.mult)

            # ---- wo + residual + rmsnorm2 ----
            wo_t = wpool.tile([128, 8, D], bf16, tag="wmat")
            nc.sync.dma_start(out=wo_t, in_=wo[:, :].rearrange("(a p) m -> p a m", p=128))
            e_t = const.tile([128, 8, OWN], f32)     # attn block output (pre-residual)
            h_t = const.tile([128, 8, OWN], f32)
            pss = psacc.tile([128, OWN], f32, tag="psacc")
            for mt in range(8):
                ph = ps.tile([128, OWN], f32, tag="ps")
                for kt in range(8):
                    nc.tensor.matmul(ph, wo_t[:, kt, mt * 128:(mt + 1) * 128],
                                     attn_bf[:, kt, :], start=(kt == 0), stop=(kt == 7))
                nc.vector.tensor_copy(out=e_t[:, mt, :], in_=ph)
                nc.vector.tensor_tensor(out=h_t[:, mt, :], in0=ph, in1=selTo_t[:, mt, :], op=op.add)
                sq = sb.tile([128, OWN], f32r, tag="sq")
                nc.vector.tensor_tensor(out=sq, in0=h_t[:, mt, :],
                                        in1=h_t[:, mt, :], op=op.mult)
                nc.tensor.matmul(pss, ones_t[:, :], sq,
                                 start=(mt == 0), stop=(mt == 7))
            ln_t = const.tile([128, OWN], f32)
            nc.scalar.activation(out=ln_t, in_=pss, func=AF.Ln,
                                 bias=eps_t[:, 0:1], scale=1.0 / D)
            rs2_t = const.tile([128, OWN], f32)
            nc.scalar.activation(out=rs2_t, in_=ln_t, func=AF.Exp, scale=-0.5)
            m_bf = const.tile([128, 8, OWN], bf16)
            for mt in range(8):
                nc.vector.tensor_tensor(out=m_bf[:, mt, :], in0=h_t[:, mt, :],
                                        in1=rs2_t, op=op.mult)

            # ---- MLP: gate (silu) then up (prod), then down ----
            sg_t = const.tile([128, 32, OWN], bf16, tag="selT_sg")
            for og in range(8):
                ch = stream.tile([128, 8, 512], bf16, tag="wgu")
                nc.gpsimd.dma_start(out=ch, in_=wg[:, og * 512:(og + 1) * 512]
                                    .rearrange("(a p) m -> p a m", p=128))
                for o4 in range(4):
                    pg = ps.tile([128, OWN], f32, tag="ps")
                    for kt in range(8):
                        nc.tensor.matmul(pg, ch[:, kt, o4 * 128:(o4 + 1) * 128],
                                         m_bf[:, kt, :], start=(kt == 0), stop=(kt == 7))
                    nc.scalar.activation(out=sg_t[:, og * 4 + o4, :], in_=pg, func=AF.Silu)
            for og in range(8):
                ch = stream.tile([128, 8, 512], bf16, tag="wgu")
                nc.gpsimd.dma_start(out=ch, in_=wu[:, og * 512:(og + 1) * 512]
                                    .rearrange("(a p) m -> p a m", p=128))
                for o4 in range(4):
                    pu = ps.tile([128, OWN], f32, tag="ps")
                    for kt in range(8):
                        nc.tensor.matmul(pu, ch[:, kt, o4 * 128:(o4 + 1) * 128],
                                         m_bf[:, kt, :], start=(kt == 0), stop=(kt == 7))
                    ot = og * 4 + o4
                    ub = sb.tile([128, OWN], bf16, tag="ub")
                    nc.vector.tensor_copy(out=ub, in_=pu)
                    nc.gpsimd.tensor_tensor(out=sg_t[:, ot, :], in0=ub,
                                            in1=sg_t[:, ot, :], op=op.mult)
            for mt in range(8):
                wdm = stream.tile([128, 32, 128], bf16, tag="wdh", bufs=2)
                nc.gpsimd.dma_start(out=wdm, in_=wd[:, mt * 128:(mt + 1) * 128]
                                    .rearrange("(a p) m -> p a m", p=128))
                pd = psacc.tile([128, OWN], f32, tag="psacc")
                for kt2 in range(32):
                    nc.tensor.matmul(pd, wdm[:, kt2, :], sg_t[:, kt2, :],
                                     start=(kt2 == 0), stop=(kt2 == 31))
                t1 = sb.tile([128, OWN], f32, tag="cmb1")
                nc.vector.tensor_tensor(out=t1, in0=pd, in1=e_t[:, mt, :], op=op.add)
                t2 = sb.tile([128, OWN], f32, tag="cmb2")
                nc.gpsimd.tensor_tensor(out=t2, in0=t1, in1=gbc_t, op=op.mult)
                uT = sb.tile([128, OWN], f32, tag="cmb3")
                nc.gpsimd.tensor_tensor(out=uT, in0=t2, in1=selTo_t[:, mt, :], op=op.add)
                updo = sb.tile([128, 2, 128], f32, tag="updo")
                for qt in range(2):
                    pt = ps.tile([128, 128], f32, tag="ps")
                    nc.tensor.transpose(pt, uT[:, qt * 128:(qt + 1) * 128], idf_t)
                    nc.vector.tensor_copy(out=updo[:, qt, :], in_=pt)
                nc.sync.dma_start(
                    out=upd[:, mt * 128:(mt + 1) * 128]
                    .rearrange("(q p) c -> p q c", p=128),
                    in_=updo)
    nc.compile()
    return nc


# --------------------------------------------------------------------------
# entry point
# --------------------------------------------------------------------------
def kernel(**inputs):
    from concourse.bass_utils import run_bass_kernel_spmd

    cores, main_aux, hs = _stage(inputs)
    if 'nc' not in _CACHE:
        _CACHE['nc'] = _build()
    nc = _CACHE['nc']

    names = ['hidT', 'fc1', 'fc1b', 'fc2w', 'fc2b', 'ttarg', 'selT', 'selTo',
             'cosT', 'sinT', 'rscol', 'm01', 'gown', 'identf',
             'wq', 'wk', 'wv', 'wo', 'wg', 'wu', 'wd']
    in_maps = [{n: ci[n] for n in names} for ci in cores]
    res = run_bass_kernel_spmd(nc, in_maps, core_ids=list(range(8)),
                               trace=bool(int(__import__('os').environ.get('KERNEL_TRACE', '0'))))
    _CACHE['last_result'] = res

    out = hs.copy()
    bce_total = 0.0
    for c in range(8):
        r = cores[c]['_row']
        out[r][cores[c]['_own_tok']] = res.results[c]['upd']
        bce_total += float(res.results[c]['bce'].sum())
    total_aux = np.float32(main_aux + (bce_total / (B * T)) * PRED_W)
    return out, total_aux


# revision 40
# speedup vs baseline: 1.2514x; 1.0116x over previous
"""Mixture-of-Depths layer on 8 Trainium2 NeuronCores.

Strategy: 4 batch rows x 2-way token split of the 512 routed tokens per row.
Host does routing (scores/top-k/sort/gate: 0.04% of FLOPs) and input staging
(transposes/casts/weight folding); each core runs the full decoder block for
its 256 tokens (k/v computed for all 512), plus 1/8 of the aux-loss predictor
GEMM. Device math is bf16 matmuls with f32 accumulation/residuals.
Output assembly (scatter of updated rows into the full tensor) is host-side.
"""

import numpy as np
import ml_dtypes

BF16 = ml_dtypes.bfloat16
F32 = np.float32

B, T, D, F = 4, 4096, 1024, 4096
NH, HD = 16, 64
K = 512            # routed tokens per row (T * 0.125)
OWN = 256          # q-tokens per core
TPC = 2048         # predictor tokens per core (B*T/8)
AUX_W = 0.01
PRED_W = 0.01
EPS = 1e-6

_CACHE = {}


# --------------------------------------------------------------------------
# host staging
# --------------------------------------------------------------------------
def _make_qperm():
    h16 = np.arange(NH)[:, None]
    first = (h16 * HD + np.arange(32)[None, :]).ravel()
    second = (h16 * HD + 32 + np.arange(32)[None, :]).ravel()
    return np.concatenate([first, second])


def _np_bce_mean(x, t):
    x = x.astype(np.float64)
    t = t.astype(np.float64)
    return np.mean(np.maximum(x, 0.0) - x * t + np.log1p(np.exp(-np.abs(x))))


def _stage(inp):
    hs = np.ascontiguousarray(inp['hidden_states'], dtype=F32)
    scores = (hs @ inp['router_w'].astype(F32)).astype(F32)
    idx = np.sort(np.argpartition(scores, T - K, axis=-1)[:, T - K:], axis=-1)
    tgt = np.zeros((B, T), F32)
    np.put_along_axis(tgt, idx, 1.0, axis=1)
    main_aux = _np_bce_mean(scores, tgt) * AUX_W
    gate = (1.0 / (1.0 + np.exp(-np.take_along_axis(scores, idx, axis=1)))).astype(F32)

    ln1 = inp['ln1_g'][:, None].astype(F32)
    ln2 = inp['ln2_g'][:, None].astype(F32)
    qperm = _make_qperm()
    wq_p = np.ascontiguousarray((ln1 * inp['wq'])[:, qperm]).astype(BF16)
    wk_p = np.ascontiguousarray((ln1 * inp['wk'])[:, qperm]).astype(BF16)
    wv_e = (ln1 * inp['wv']).astype(BF16)
    wo_bf = inp['wo'].astype(BF16)
    wg_e = (ln2 * inp['w_gate']).astype(BF16)
    wu_e = (ln2 * inp['w_up']).astype(BF16)
    wd_bf = inp['w_down'].astype(BF16)
    fc1_bf = inp['fc1_w'].astype(BF16)
    hidT_bf = np.ascontiguousarray(hs.reshape(B * T, D).T).astype(BF16)

    inv = (1.0 / (10000.0 ** (np.arange(0, HD, 2, dtype=F32) / HD))).astype(F32)
    ident = np.eye(128, dtype=F32)

    cores = []
    for c in range(8):
        r, h = c // 2, c % 2
        perm = np.arange(K) if h == 0 else np.concatenate(
            [np.arange(OWN, K), np.arange(OWN)])
        posp = idx[r][perm]
        pos_f = posp.astype(F32)
        sel_p = hs[r][posp]                                     # [512, D]
        rs = (1.0 / np.sqrt((sel_p ** 2).mean(axis=1) + EPS)).astype(F32)
        ang = pos_f[None, :] * inv[(np.arange(128) % 32)][:, None]   # [128,512]
        cos_t = (np.cos(ang) * rs[None, :]).astype(F32)
        sin_t = (np.sin(ang) * rs[None, :]).astype(F32)
        rs_col = np.ascontiguousarray(rs.reshape(4, 128).T)     # [128, 4]
        # transposed mask tiles: m01T[j, p, q] = 1 if pos[kt=j*128+p] <= pos[qt=q]
        M01T = (posp[:, None] <= posp[None, :OWN]).astype(F32).reshape(4, 128, OWN)
        tslice = slice(c * TPC, (c + 1) * TPC)
        tt = tgt.reshape(B * T)[tslice]
        cores.append(dict(
            hidT=np.ascontiguousarray(hidT_bf[:, tslice]),
            selT=np.ascontiguousarray(sel_p.T).astype(BF16),
            selTo=np.ascontiguousarray(sel_p[:OWN].T).astype(F32),
            cosT=cos_t, sinT=sin_t, rscol=rs_col,
            m01=np.ascontiguousarray(M01T).astype(BF16),
            ttarg=np.ascontiguousarray(tt.reshape(16, 128).T).astype(F32),
            gown=gate[r][perm[:OWN]].reshape(1, OWN).astype(F32),
            fc1b=np.ascontiguousarray(
                np.broadcast_to(inp['fc1_b'].astype(F32), (128, 256))),
            fc2w=np.ascontiguousarray(
                np.broadcast_to(inp['fc2_w'], (128, 256))).astype(BF16),
            fc2b=np.full((128, 1), np.float32(inp['fc2_b']), F32),
            identf=ident,
            wq=wq_p, wk=wk_p, wv=wv_e, wo=wo_bf,
            wg=wg_e, wu=wu_e, wd=wd_bf, fc1=fc1_bf,
        ))
        cores[-1]['_own_tok'] = posp[:OWN]
        cores[-1]['_row'] = r
    return cores, main_aux, hs


# --------------------------------------------------------------------------
# device program
# --------------------------------------------------------------------------
def _build():
    import concourse.bass as bass
    import concourse.bacc as bacc
    import concourse.mybir as mybir
    import concourse.tile as tile
    from concourse.alu_op_type import AluOpType as op
    dt = mybir.dt
    AF = mybir.ActivationFunctionType
    AX = mybir.AxisListType.X
    SafeTileContext = tile.TileContext

    nc = bacc.Bacc("TRN2")
    I = {}
    def di(name, shape, d=dt.bfloat16):
        I[name] = nc.dram_tensor(name, shape, d, kind="ExternalInput")
        return I[name]

    hidT = di('hidT', [D, TPC])
    fc1 = di('fc1', [D, 256])
    fc1b = di('fc1b', [128, 256], dt.float32)
    fc2w = di('fc2w', [128, 256])
    fc2b = di('fc2b', [128, 1], dt.float32)
    ttarg = di('ttarg', [128, 16], dt.float32)
    selT = di('selT', [D, K])
    selTo = di('selTo', [D, OWN], dt.float32)
    cosT = di('cosT', [128, K], dt.float32)
    sinT = di('sinT', [128, K], dt.float32)
    rscol = di('rscol', [128, 4], dt.float32)
    m01 = di('m01', [4, 128, OWN])
    gown = di('gown', [1, OWN], dt.float32)
    identf = di('identf', [128, 128], dt.float32)
    wq = di('wq', [D, D]); wk = di('wk', [D, D])
    wv = di('wv', [D, D]); wo = di('wo', [D, D])
    wg = di('wg', [D, F]); wu = di('wu', [D, F])
    wd = di('wd', [F, D])

    upd = nc.dram_tensor('upd', [OWN, D], dt.float32, kind="ExternalOutput")
    bce = nc.dram_tensor('bce', [128, 1], dt.float32, kind="ExternalOutput")

    f32, bf16 = dt.float32, dt.bfloat16
    f32r = dt.float32r

    with SafeTileContext(nc) as tc:
        import contextlib
        ctx = contextlib.ExitStack()
        with ctx:
            const = ctx.enter_context(tc.tile_pool(name="const", bufs=1))
            sb = ctx.enter_context(tc.tile_pool(name="sb", bufs=2))
            wpool = ctx.enter_context(tc.tile_pool(name="wpool", bufs=2))
            stream = ctx.enter_context(tc.tile_pool(name="stream", bufs=2))
            ps = ctx.enter_context(tc.tile_pool(name="ps", bufs=6, space="PSUM"))
            psacc = ctx.enter_context(tc.tile_pool(name="psacc", bufs=2, space="PSUM"))

            # ---- constants / small loads ----
            selT_t = const.tile([128, 8, K], bf16, tag="selT_sg")
            nc.sync.dma_start(out=selT_t, in_=selT[:, :].rearrange("(a p) t -> p a t", p=128))
            selTo_t = const.tile([128, 8, OWN], f32)
            nc.sync.dma_start(out=selTo_t, in_=selTo[:, :].rearrange("(a p) t -> p a t", p=128))
            cos_t = const.tile([128, K], f32)
            nc.sync.dma_start(out=cos_t, in_=cosT[:, :])
            sin_t = const.tile([128, K], f32)
            nc.sync.dma_start(out=sin_t, in_=sinT[:, :])
            rs_t = const.tile([128, 4], f32)
            nc.sync.dma_start(out=rs_t, in_=rscol[:, :])
            m01_t = const.tile([128, 4, OWN], bf16)
            nc.sync.dma_start(out=m01_t, in_=m01[:, :, :].rearrange("j p q -> p j q"))
            fc1b_t = const.tile([128, 256], f32)
            nc.sync.dma_start(out=fc1b_t, in_=fc1b[:, :])
            fc2w_t = const.tile([128, 256], bf16)
            nc.sync.dma_start(out=fc2w_t, in_=fc2w[:, :])
            fc2b_t = const.tile([128, 1], f32)
            nc.sync.dma_start(out=fc2b_t, in_=fc2b[:, :])
            ttarg_t = const.tile([128, 16], f32)
            nc.sync.dma_start(out=ttarg_t, in_=ttarg[:, :])
            onesb_t = const.tile([128, 128], bf16)
            nc.vector.memset(onesb_t, 1.0)
            idf_t = const.tile([128, 128], f32)
            nc.sync.dma_start(out=idf_t, in_=identf[:, :])
            g_ap = gown[:, :]
            gbc_t = const.tile([128, OWN], f32)
            nc.sync.dma_start(out=gbc_t, in_=bass.AP(
                tensor=g_ap.tensor, offset=g_ap.offset, ap=[[0, 128], g_ap.ap[-1]]))
            eps_t = const.tile([128, 1], f32)
            nc.vector.memset(eps_t, EPS)
            ones_f = const.tile([128, 128], f32)
            nc.vector.memset(ones_f, 1.0)
            ones_t = const.tile([128, 128], f32r)
            nc.vector.tensor_copy(out=ones_t, in_=ones_f)

            # ---- predictor pieces ----
            # fc1 matmuls are emitted inside the attention loop (dense PE
            # filler keeps HAM warm); gelu/fc2/bce run in the down phase
            # where ACT/DVE are idle (also isolates act-table switches).
            fc1_t = const.tile([128, 8, 256], bf16)
            nc.sync.dma_start(out=fc1_t, in_=fc1[:, :].rearrange("(a p) n -> p a n", p=128))
            logit_t = const.tile([128, 16], f32)
            pre_t = const.tile([128, 16, 256], bf16)

            def pred_fc1(grp):
                hid_t = stream.tile([128, 8, 512], bf16, tag="wdh", bufs=2)
                nc.gpsimd.dma_start(
                    out=hid_t,
                    in_=hidT[:, grp * 512:(grp + 1) * 512]
                    .rearrange("(a p) t -> p a t", p=128))
                for mi in range(4):
                    mt = grp * 4 + mi
                    pp = psacc.tile([128, 256], f32, tag="psacc")
                    for kt in range(8):
                        nc.tensor.matmul(pp, hid_t[:, kt, mi * 128:(mi + 1) * 128],
                                         fc1_t[:, kt, :], start=(kt == 0), stop=(kt == 7))
                    nc.vector.tensor_tensor(out=pre_t[:, mt, :], in0=pp,
                                            in1=fc1b_t, op=op.add)

            def pred_post(mt):
                gel = sb.tile([128, 256], bf16, tag="pred_gel")
                nc.scalar.activation(out=gel, in_=pre_t[:, mt, :], func=AF.Gelu_apprx_tanh)
                fm = sb.tile([128, 256], f32, tag="pred_fm")
                nc.vector.tensor_tensor(out=fm, in0=gel, in1=fc2w_t, op=op.mult)
                nc.vector.tensor_reduce(out=logit_t[:, mt:mt + 1], in_=fm,
                                        axis=AX, op=op.add)

            def pred_tail():
                # logits += fc2b ; bce = relu(x) - x*t + log1p(exp(-|x|))
                nc.vector.tensor_scalar(logit_t, logit_t, fc2b_t[:, 0:1], None, op0=op.add)
                xt_t = const.tile([128, 16], f32)
                nc.vector.tensor_tensor(out=xt_t, in0=logit_t, in1=ttarg_t, op=op.mult)
                r0_t = const.tile([128, 16], f32)
                nc.vector.scalar_tensor_tensor(out=r0_t, in0=logit_t, scalar=0.0,
                                               in1=xt_t, op0=op.max, op1=op.subtract)
                ab_t = const.tile([128, 16], f32)
                nc.scalar.activation(out=ab_t, in_=logit_t, func=AF.Abs)
                ex_t = const.tile([128, 16], f32)
                nc.scalar.activation(out=ex_t, in_=ab_t, func=AF.Exp, scale=-1.0)
                sp_t = const.tile([128, 16], f32)
                nc.scalar.activation(out=sp_t, in_=ex_t, func=AF.Ln, bias=1.0)
                be_t = const.tile([128, 16], f32)
                nc.vector.tensor_tensor(out=be_t, in0=r0_t, in1=sp_t, op=op.add)
                bce_t = const.tile([128, 1], f32)
                nc.vector.tensor_reduce(out=bce_t, in_=be_t, axis=AX, op=op.add)
                nc.sync.dma_start(out=bce[:, :], in_=bce_t)

            # ---- q/k projections + rope ----
            wq_t = wpool.tile([128, 8, D], bf16, tag="wmat")
            nc.sync.dma_start(out=wq_t, in_=wq[:, :].rearrange("(a p) m -> p a m", p=128))
            wk_t = wpool.tile([128, 8, D], bf16, tag="wmat")
            nc.sync.dma_start(out=wk_t, in_=wk[:, :].rearrange("(a p) m -> p a m", p=128))
            q_bf = const.tile([128, 8, OWN], bf16)
            k_bf = const.tile([128, 8, K], bf16)

            def qk_proj(w_t, out_t, n):
                for i in range(4):
                    pa = ps.tile([128, n], f32, tag="ps")
                    pb = ps.tile([128, n], f32, tag="ps")
                    for kt in range(8):
                        nc.tensor.matmul(pa, w_t[:, kt, i * 128:(i + 1) * 128],
                                         selT_t[:, kt, 0:n], start=(kt == 0), stop=(kt == 7))
                    for kt in range(8):
                        nc.tensor.matmul(pb, w_t[:, kt, (i + 4) * 128:(i + 5) * 128],
                                         selT_t[:, kt, 0:n], start=(kt == 0), stop=(kt == 7))
                    t0 = sb.tile([128, n], f32, tag="rope0")
                    t1 = sb.tile([128, n], f32, tag="rope1")
                    nc.vector.tensor_tensor(out=t0, in0=pa, in1=cos_t[:, 0:n], op=op.mult)
                    nc.vector.tensor_tensor(out=t1, in0=pb, in1=sin_t[:, 0:n], op=op.mult)
                    nc.gpsimd.tensor_tensor(out=out_t[:, i, :], in0=t0, in1=t1, op=op.subtract)
                    t2 = sb.tile([128, n], f32, tag="rope0")
                    t3 = sb.tile([128, n], f32, tag="rope1")
                    nc.vector.tensor_tensor(out=t2, in0=pb, in1=cos_t[:, 0:n], op=op.mult)
                    nc.vector.tensor_tensor(out=t3, in0=pa, in1=sin_t[:, 0:n], op=op.mult)
                    nc.gpsimd.tensor_tensor(out=out_t[:, i + 4, :], in0=t2, in1=t3, op=op.add)

            qk_proj(wq_t, q_bf, OWN)
            pred_fc1(0)
            qk_proj(wk_t, k_bf, K)
            pred_fc1(1)

            # ---- v projection (token-major) ----
            wv_t = wpool.tile([128, 8, D], bf16, tag="wmat")
            nc.sync.dma_start(out=wv_t, in_=wv[:, :].rearrange("(a p) m -> p a m", p=128))
            v_bf = const.tile([128, 4, D], bf16)
            for j in range(4):
                for half in range(2):
                    pv = ps.tile([128, 512], f32, tag="ps")
                    for kt in range(8):
                        nc.tensor.matmul(pv, selT_t[:, kt, j * 128:(j + 1) * 128],
                                         wv_t[:, kt, half * 512:(half + 1) * 512],
                                         start=(kt == 0), stop=(kt == 7))
                    nc.vector.tensor_scalar(v_bf[:, j, half * 512:(half + 1) * 512],
                                            pv, rs_t[:, j:j + 1], None, op0=op.mult)

            # ---- attention, S computed transposed: P_T[kt, qt] ----
            # exp -> mask (DVE) -> row-sums via ones-matmul (PE) -> PV directly
            # from P_T (no PE transposes); normalization folded into the
            # PSUM->SBUF copy of the PV output (free-dim op).
            attn_bf = const.tile([128, 8, OWN], bf16)
            for hp in range(8):
                po = ps.tile([128, OWN], f32, tag="ps")
                rcps = []
                for sub in range(2):
                    hh = 2 * hp + sub
                    rstrip, tb = hh % 4, hh // 4
                    lo = 32 * rstrip
                    pt = sb.tile([128, 4, OWN], bf16, tag="pbf", bufs=4)
                    for half in range(2):
                        st = ps.tile([128, 2, OWN], f32, tag="ps")
                        for jj in range(2):
                            j = half * 2 + jj
                            nc.tensor.matmul(
                                st[:, jj, :],
                                k_bf[lo:lo + 32, tb, j * 128:(j + 1) * 128],
                                q_bf[lo:lo + 32, tb, :], start=True, stop=False,
                                tile_position=(lo, 0))
                            nc.tensor.matmul(
                                st[:, jj, :],
                                k_bf[lo:lo + 32, tb + 4, j * 128:(j + 1) * 128],
                                q_bf[lo:lo + 32, tb + 4, :], start=False, stop=True,
                                tile_position=(lo, 0))
                        nc.scalar.activation(out=pt[:, half * 2:half * 2 + 2, :],
                                             in_=st, func=AF.Exp, scale=0.125)
                        nc.vector.tensor_tensor(
                            out=pt[:, half * 2:half * 2 + 2, :],
                            in0=pt[:, half * 2:half * 2 + 2, :],
                            in1=m01_t[:, half * 2:half * 2 + 2, :], op=op.mult)
                    pssum = ps.tile([128, OWN], f32, tag="ps")
                    for j in range(4):
                        nc.tensor.matmul(pssum, onesb_t, pt[:, j, :],
                                         start=(j == 0), stop=(j == 3))
                    rcpb = sb.tile([128, OWN], f32, tag="rcp", bufs=3)
                    nc.vector.reciprocal_approx_fast(out=rcpb, in_=pssum)
                    rcps.append(rcpb)
                    for j in range(4):
                        nc.tensor.matmul(po[64 * sub:64 * sub + 64, :],
                                         v_bf[:, j, 64 * hh:64 * hh + 64],
                                         pt[:, j, :],
                                         start=(j == 0), stop=(j == 3),
                                         tile_position=(0, 64 * sub))
                for sub in range(2):
                    sl = slice(64 * sub, 64 * sub + 64)
                    nc.vector.tensor_tensor(out=attn_bf[sl, hp, :], in0=po[sl, :],
                                            in1=rcps[sub][sl, :], op=op.mult)
                if hp == 0:
                    pred_fc1(2)
                elif hp == 1:
                    pred_fc1(3)

            # ---- wo + residual + rmsnorm2 ----
            wo_t = wpool.tile([128, 8, D], bf16, tag="wmat")
            nc.sync.dma_start(out=wo_t, in_=wo[:, :].rearrange("(a p) m -> p a m", p=128))
            e_t = const.tile([128, 8, OWN], f32)     # attn block output (pre-residual)
            h_t = const.tile([128, 8, OWN], f32)
            pss = psacc.tile([128, OWN], f32, tag="psacc")
            for mt in range(8):
                ph = ps.tile([128, OWN], f32, tag="ps")
                for kt in range(8):
                    nc.tensor.matmul(ph, wo_t[:, kt, mt * 128:(mt + 1) * 128],
                                     attn_bf[:, kt, :], start=(kt == 0), stop=(kt == 7))
                nc.vector.tensor_copy(out=e_t[:, mt, :], in_=ph)
                nc.vector.tensor_tensor(out=h_t[:, mt, :], in0=ph, in1=selTo_t[:, mt, :], op=op.add)
                sq = sb.tile([128, OWN], f32r, tag="sq")
                nc.vector.tensor_tensor(out=sq, in0=h_t[:, mt, :],
                                        in1=h_t[:, mt, :], op=op.mult)
                nc.tensor.matmul(pss, ones_t[:, :], sq,
                                 start=(mt == 0), stop=(mt == 7))
            ln_t = const.tile([128, OWN], f32)
            nc.scalar.activation(out=ln_t, in_=pss, func=AF.Ln,
                                 bias=eps_t[:, 0:1], scale=1.0 / D)
            rs2_t = const.tile([128, OWN], f32)
            nc.scalar.activation(out=rs2_t, in_=ln_t, func=AF.Exp, scale=-0.5)
            m_bf = const.tile([128, 8, OWN], bf16)
            for mt in range(8):
                nc.vector.tensor_tensor(out=m_bf[:, mt, :], in0=h_t[:, mt, :],
                                        in1=rs2_t, op=op.mult)

            # ---- MLP: gate (silu) then up (prod), then down ----
            sg_t = const.tile([128, 32, OWN], bf16, tag="selT_sg")
            for og in range(8):
                ch = stream.tile([128, 8, 512], bf16, tag="wgu")
                nc.gpsimd.dma_start(out=ch, in_=wg[:, og * 512:(og + 1) * 512]
                                    .rearrange("(a p) m -> p a m", p=128))
                for o4 in range(4):
                    pg = ps.tile([128, OWN], f32, tag="ps")
                    for kt in range(8):
                        nc.tensor.matmul(pg, ch[:, kt, o4 * 128:(o4 + 1) * 128],
                                         m_bf[:, kt, :], start=(kt == 0), stop=(kt == 7))
                    nc.scalar.activation(out=sg_t[:, og * 4 + o4, :], in_=pg, func=AF.Silu)
            for og in range(8):
                ch = stream.tile([128, 8, 512], bf16, tag="wgu")
                nc.gpsimd.dma_start(out=ch, in_=wu[:, og * 512:(og + 1) * 512]
                                    .rearrange("(a p) m -> p a m", p=128))
                for o4 in range(4):
                    pu = ps.tile([128, OWN], f32, tag="ps")
                    for kt in range(8):
                        nc.tensor.matmul(pu, ch[:, kt, o4 * 128:(o4 + 1) * 128],
                                         m_bf[:, kt, :], start=(kt == 0), stop=(kt == 7))
                    ot = og * 4 + o4
                    ub = sb.tile([128, OWN], bf16, tag="ub")
                    nc.vector.tensor_copy(out=ub, in_=pu)
                    nc.gpsimd.tensor_tensor(out=sg_t[:, ot, :], in0=ub,
                                            in1=sg_t[:, ot, :], op=op.mult)
            for mt in range(8):
                wdm = stream.tile([128, 32, 128], bf16, tag="wdh", bufs=2)
                nc.gpsimd.dma_start(out=wdm, in_=wd[:, mt * 128:(mt + 1) * 128]
                                    .rearrange("(a p) m -> p a m", p=128))
                pd = psacc.tile([128, OWN], f32, tag="psacc")
                for kt2 in range(32):
                    nc.tensor.matmul(pd, wdm[:, kt2, :], sg_t[:, kt2, :],
                                     start=(kt2 == 0), stop=(kt2 == 31))
                t1 = sb.tile([128, OWN], f32, tag="cmb1")
                nc.vector.tensor_tensor(out=t1, in0=pd, in1=e_t[:, mt, :], op=op.add)
                t2 = sb.tile([128, OWN], f32, tag="cmb2")
                nc.gpsimd.tensor_tensor(out=t2, in0=t1, in1=gbc_t, op=op.mult)
                uT = sb.tile([128, OWN], f32, tag="cmb3")
                nc.gpsimd.tensor_tensor(out=uT, in0=t2, in1=selTo_t[:, mt, :], op=op.add)
                updo = sb.tile([128, 2, 128], f32, tag="updo")
                for qt in range(2):
                    pt = ps.tile([128, 128], f32, tag="ps")
                    nc.tensor.transpose(pt, uT[:, qt * 128:(qt + 1) * 128], idf_t)
                    nc.vector.tensor_copy(out=updo[:, qt, :], in_=pt)
                nc.sync.dma_start(
                    out=upd[:, mt * 128:(mt + 1) * 128]
                    .rearrange("(q p) c -> p q c", p=128),
                    in_=updo)
                pred_post(2 * mt)
                pred_post(2 * mt + 1)
            pred_tail()
    nc.compile()
    return nc


# --------------------------------------------------------------------------
# entry point
# --------------------------------------------------------------------------
def kernel(**inputs):
    from concourse.bass_utils import run_bass_kernel_spmd

    cores, main_aux, hs = _stage(inputs)
    if 'nc' not in _CACHE:
        _CACHE['nc'] = _build()
    nc = _CACHE['nc']

    names = ['hidT', 'fc1', 'fc1b', 'fc2w', 'fc2b', 'ttarg', 'selT', 'selTo',
             'cosT', 'sinT', 'rscol', 'm01', 'gown', 'identf',
             'wq', 'wk', 'wv', 'wo', 'wg', 'wu', 'wd']
    in_maps = [{n: ci[n] for n in names} for ci in cores]
    res = run_bass_kernel_spmd(nc, in_maps, core_ids=list(range(8)),
                               trace=bool(int(__import__('os').environ.get('KERNEL_TRACE', '0'))))
    _CACHE['last_result'] = res

    out = hs.copy()
    bce_total = 0.0
    for c in range(8):
        r = cores[c]['_row']
        out[r][cores[c]['_own_tok']] = res.results[c]['upd']
        bce_total += float(res.results[c]['bce'].sum())
    total_aux = np.float32(main_aux + (bce_total / (B * T)) * PRED_W)
    return out, total_aux
